# revision 1
# baseline (speedup 1.0000x reference)
"""GraphTransformerEncoder (8-layer TransformerConv + BN + ReLU + mean-pool)
on 8 Trainium2 NeuronCores via Bass/Tile.

Sharding: graph-parallel. Core c owns graphs [8c, 8c+8) -> a contiguous node
range (batch is sorted). Edges are owned by the core of their dst node, sorted
by dst, and packed into fixed-stride per-128-node-block slots. Per layer each
core computes Q/K/V/root projections for its nodes, AllGathers the K|V table
(bf16, node-rows), DMA-gathers K|V rows for its edges' src nodes, computes the
edge softmax via segment-indicator matmuls, applies BN (global stats via a tiny
AllReduce) + ReLU, and mean-pools its graphs.
"""

import numpy as np
import ml_dtypes

import concourse.bass as bass
import concourse.bacc as bacc
import concourse.mybir as mybir
import concourse.tile as tile
from concourse import library_config
from contextlib import ExitStack

BF = mybir.dt.bfloat16
F32 = mybir.dt.float32
I16 = mybir.dt.int16
AF = mybir.ActivationFunctionType

# problem constants
N, E, F, H, C, L, B = 10000, 160000, 128, 8, 64, 8, 64
D = H * C  # 512
BN_EPS = 1e-5

# sharding constants
NCORE = 8
GPC = B // NCORE        # graphs per core = 8
NT = 11                 # node blocks (128) per core
NLOC = NT * 128         # 1408 node slots per core
NCH = 20                # edge chunks (128) per node block
ES = NCH * 128          # 2560 edge slots per node block
CHUNKS = NT * NCH       # 220
SLOTS = NT * ES         # 28160 edge slots per core
GI = 512                # indices per dma_gather (4 chunks)
GPB = ES // GI          # gathers per block = 5
NGA = SLOTS // GI       # gathers per core = 55
KVROWS = NCORE * NLOC   # 11264 rows in the all-gathered KV table


def _to_bf(a):
    return np.asarray(a, dtype=np.float32).astype(ml_dtypes.bfloat16)


def _build_nc():
    nc = bacc.Bacc("TRN2", num_devices=NCORE,
                  target_bir_lowering=False, debug=False)
    rg = [list(range(NCORE))]

    # ---- I/O -----------------------------------------------------------
    XT = nc.dram_tensor("XT", [128, NLOC], BF, kind="ExternalInput")
    W0 = nc.dram_tensor("W0", [128, 4 * 512], BF, kind="ExternalInput")
    WR = nc.dram_tensor("WR", [7 * 2048, 512], BF, kind="ExternalInput")
    BIAS = nc.dram_tensor("BIAS", [1, 8 * 2048], BF, kind="ExternalInput")
    GAM = nc.dram_tensor("GAM", [1, 8 * 512], F32, kind="ExternalInput")
    BET = nc.dram_tensor("BET", [1, 8 * 512], F32, kind="ExternalInput")
    IDX = nc.dram_tensor("IDX", [128, NGA * (GI // 16)], I16, kind="ExternalInput")
    DSTC = nc.dram_tensor("DSTC", [128, CHUNKS], F32, kind="ExternalInput")
    IOTAF = nc.dram_tensor("IOTAF", [128, 128], BF, kind="ExternalInput")
    IOTAC = nc.dram_tensor("IOTAC", [128, 1], F32, kind="ExternalInput")
    ONES1 = nc.dram_tensor("ONES1", [1, 128], BF, kind="ExternalInput")
    IDENTF = nc.dram_tensor("IDENTF", [128, 128], F32, kind="ExternalInput")
    IDENTB = nc.dram_tensor("IDENTB", [128, 128], BF, kind="ExternalInput")
    MASK = nc.dram_tensor("MASK", [128, NT], F32, kind="ExternalInput")
    SPOOL = nc.dram_tensor("SPOOL", [128, NT * GPC], BF, kind="ExternalInput")
    CNTR = nc.dram_tensor("CNTR", [GPC, 1], F32, kind="ExternalInput")
    OUT = nc.dram_tensor("POOLED", [GPC, L * 512], F32, kind="ExternalOutput")

    with tile.TileContext(nc) as tc, ExitStack() as ctx:
        sb1 = ctx.enter_context(tc.tile_pool(name="sb1", bufs=1))
        sbh = ctx.enter_context(tc.tile_pool(name="sbh", bufs=2))
        sbw = ctx.enter_context(tc.tile_pool(name="sbw", bufs=2))
        sbs = ctx.enter_context(tc.tile_pool(name="sbs", bufs=3))
        sbg = ctx.enter_context(tc.tile_pool(name="sbg", bufs=3))
        sbm = ctx.enter_context(tc.tile_pool(name="sbm", bufs=2))
        ps = ctx.enter_context(tc.tile_pool(name="ps", bufs=1, space="PSUM"))
        dram = ctx.enter_context(tc.tile_pool(name="dram", bufs=2, space="DRAM"))

        def load1(src, shape, dtype, name):
            t = sb1.tile(shape, dtype, name=name)
            nc.sync.dma_start(out=t[:], in_=src[:])
            return t

        iota_f = load1(IOTAF, [128, 128], BF, "iota_f")
        iota_c = load1(IOTAC, [128, 1], F32, "iota_c")
        ones1 = load1(ONES1, [1, 128], BF, "ones1")
        identf = load1(IDENTF, [128, 128], F32, "identf")
        identb = load1(IDENTB, [128, 128], BF, "identb")
        idx_sb = load1(IDX, [128, NGA * (GI // 16)], I16, "idx_sb")
        dstc_sb = load1(DSTC, [128, CHUNKS], F32, "dstc_sb")
        mask_sb = load1(MASK, [128, NT], F32, "mask_sb")
        spool_sb = load1(SPOOL, [128, NT * GPC], BF, "spool_sb")
        cntr_sb = load1(CNTR, [GPC, 1], F32, "cntr_sb")
        gam_sb = load1(GAM, [1, 8 * 512], F32, "gam_sb")
        bet_sb = load1(BET, [1, 8 * 512], F32, "bet_sb")

        czero = sb1.tile([128, 1], F32, name="czero")
        nc.vector.memset(czero[:], 0.0)
        ceps = sb1.tile([128, 1], F32, name="ceps")
        nc.vector.memset(ceps[:], BN_EPS)
        nc.const_aps.aps[(F32, 0.0)] = czero[:]
        nc.const_aps.aps[(F32, BN_EPS)] = ceps[:]

        nc.gpsimd.load_library(library_config.mlp)

        h_cur = sbh.tile([128, 4, NLOC], BF, tag="h", name="h0")
        nc.sync.dma_start(out=h_cur[:, 0, :], in_=XT[:, :])

        for l in range(L):
            KIN = 1 if l == 0 else 4

            # -- weights for this layer -> SBUF [128, 4*KIN, 512]
            w_sb = sbw.tile([128, 4 * KIN, 512], BF, tag="w", name=f"w{l}")
            if l == 0:
                nc.sync.dma_start(
                    out=w_sb[:], in_=W0[:, :].rearrange("p (c n) -> p c n", c=4))
            else:
                nc.sync.dma_start(
                    out=w_sb[:],
                    in_=WR[(l - 1) * 2048: l * 2048, :].rearrange(
                        "(c p) n -> p c n", p=128))

            bias_sb = sbs.tile([1, 2048], BF, tag="bias", bufs=2,
                               name=f"bias{l}")
            nc.sync.dma_start(out=bias_sb[:],
                              in_=BIAS[0:1, l * 2048:(l + 1) * 2048])
            kv_loc = dram.tile([NLOC, 1024], BF, tag="kvloc", name=f"kvloc{l}")
            kv_full = dram.tile([KVROWS, 1024], BF, tag="kvfull",
                                addr_space="Shared", name=f"kvfull{l}")

            # -- projections Q, K, V  (root is computed per block later)
            Q_sb = sbm.tile([128, NT, 512], BF, tag="q", bufs=1, name=f"q{l}")
            for m in range(NT):
                kv_sb = sbg.tile([128, 1024], BF, tag="kv", name=f"kv{l}_{m}")
                for pr in range(3):  # 0=q 1=k 2=v
                    pp = ps.tile([128, 512], F32, tag="qd", bufs=2,
                                 name=f"pp{l}_{m}_{pr}")
                    for kc in range(KIN):
                        nc.tensor.matmul(
                            pp[:], lhsT=h_cur[:, kc, m * 128:(m + 1) * 128],
                            rhs=w_sb[:, pr * KIN + kc, :],
                            start=(kc == 0), stop=False)
                    nc.tensor.matmul(
                        pp[:], lhsT=ones1[:],
                        rhs=bias_sb[0:1, pr * 512:(pr + 1) * 512],
                        start=False, stop=True)
                    if pr == 0:
                        nc.scalar.activation(Q_sb[:, m, :], pp[:], AF.Copy)
                    else:
                        nc.scalar.activation(
                            kv_sb[:, (pr - 1) * 512: pr * 512], pp[:], AF.Copy)
                nc.sync.dma_start(out=kv_loc[m * 128:(m + 1) * 128, :],
                                  in_=kv_sb[:])

            # -- AllGather the K|V table
            nc.gpsimd.collective_compute(
                "AllGather", mybir.AluOpType.bypass, replica_groups=rg,
                ins=[kv_loc[:].opt()], outs=[kv_full[:].opt()])

            # -- edge stage
            statacc = sbm.tile([1, 1024], F32, tag="stat", bufs=1,
                               name=f"stat{l}")
            nc.vector.memset(statacc[:], 0.0)
            pre_sb = sbm.tile([128, NT, 512], F32, tag="pre", bufs=1,
                              name=f"pre{l}")

            for m in range(NT):
                # root projection for this block
                rt = ps.tile([128, 512], F32, tag="root", bufs=1,
                             name=f"rt{l}_{m}")
                for kc in range(KIN):
                    nc.tensor.matmul(
                        rt[:], lhsT=h_cur[:, kc, m * 128:(m + 1) * 128],
                        rhs=w_sb[:, 3 * KIN + kc, :],
                        start=(kc == 0), stop=False)
                nc.tensor.matmul(rt[:], lhsT=ones1[:],
                                 rhs=bias_sb[0:1, 3 * 512:4 * 512],
                                 start=False, stop=True)

                acc = ps.tile([128, 512], F32, tag="acc", bufs=2,
                              name=f"acc{l}_{m}")
                den = ps.tile([128, 8], F32, tag="den", bufs=1,
                              name=f"den{l}_{m}")
                for g in range(GPB):
                    gt = sbg.tile([128, GI // 128, 1024], BF, tag="g",
                                  name=f"gt{l}_{m}_{g}")
                    ga = m * GPB + g
                    nc.gpsimd.dma_gather(
                        gt[:], kv_full[:, :],
                        idx_sb[:, ga * (GI // 16):(ga + 1) * (GI // 16)],
                        GI, GI, 1024)
                    for cc in range(GI // 128):
                        ch = m * NCH + g * (GI // 128) + cc
                        first = (g == 0 and cc == 0)
                        last = (g == GPB - 1 and cc == GI // 128 - 1)

                        st_sb = sbs.tile([128, 128], BF, tag="st",
                                         name=f"st{l}_{ch}")
                        nc.gpsimd.tensor_scalar(
                            out=st_sb[:], in0=iota_f[:],
                            scalar1=dstc_sb[:, ch:ch + 1],
                            scalar2=None, op0=mybir.AluOpType.is_equal)
                        str_ps = ps.tile([128, 128], BF, tag="str", bufs=1,
                                         name=f"strp{l}_{ch}")
                        nc.tensor.transpose(str_ps[:], st_sb[:], identb[:])
                        s_sb = sbs.tile([128, 128], BF, tag="s",
                                        name=f"s{l}_{ch}")
                        nc.vector.tensor_copy(out=s_sb[:], in_=str_ps[:])

                        qd = ps.tile([128, 512], F32, tag="qd", bufs=2,
                                     name=f"qd{l}_{ch}")
                        nc.tensor.matmul(qd[:], lhsT=s_sb[:],
                                         rhs=Q_sb[:, m, :],
                                         start=True, stop=True)
                        prod = sbm.tile([128, 512], BF, tag="prod", bufs=2,
                                        name=f"prod{l}_{ch}")
                        nc.vector.tensor_mul(prod[:], qd[:],
                                             gt[:, cc, 0:512])
                        logit = sbs.tile([128, 8], F32, tag="lg",
                                         name=f"lg{l}_{ch}")
                        nc.vector.tensor_reduce(
                            logit[:],
                            prod[:].rearrange("p (h c) -> p h c", h=8),
                            mybir.AxisListType.X, mybir.AluOpType.add)
                        pf = sbs.tile([128, 8], F32, tag="pf",
                                      name=f"pf{l}_{ch}")
                        nc.scalar.activation(pf[:], logit[:], AF.Exp,
                                             scale=0.125)
                        pbf = sbs.tile([128, 8], BF, tag="p",
                                       name=f"p{l}_{ch}")
                        nc.vector.tensor_copy(out=pbf[:], in_=pf[:])
                        pv = sbm.tile([128, 512], BF, tag="pv", bufs=2,
                                      name=f"pv{l}_{ch}")
                        for h in range(H):
                            nc.scalar.activation(
                                pv[:, h * 64:(h + 1) * 64],
                                gt[:, cc, 512 + h * 64: 512 + (h + 1) * 64],
                                AF.Identity, scale=pf[:, h:h + 1])
                        nc.tensor.matmul(acc[:], lhsT=st_sb[:], rhs=pv[:],
                                         start=first, stop=last)
                        nc.tensor.matmul(den[:], lhsT=st_sb[:], rhs=pbf[:],
                                         start=first, stop=last)

                # block finalize: normalize, add root, stats
                dsb = sbs.tile([128, 8], F32, tag="dsb", name=f"dsb{l}_{m}")
                nc.scalar.activation(dsb[:], den[:], AF.Copy, bias=1e-16)
                rec = sbs.tile([128, 8], F32, tag="rec", name=f"rec{l}_{m}")
                nc.vector.reciprocal(rec[:], dsb[:])
                msg = sbm.tile([128, 512], F32, tag="msg", bufs=2,
                               name=f"msg{l}_{m}")
                for h in range(H):
                    nc.scalar.activation(
                        msg[:, h * 64:(h + 1) * 64],
                        acc[:, h * 64:(h + 1) * 64],
                        AF.Identity, scale=rec[:, h:h + 1])
                nc.vector.tensor_add(pre_sb[:, m, :], msg[:], rt[:])
                sq = sbm.tile([128, 512], F32, tag="sq", bufs=2,
                              name=f"sq{l}_{m}")
                nc.scalar.activation(sq[:], pre_sb[:, m, :], AF.Square)
                stp = ps.tile([1, 512], F32, tag="stp", bufs=1,
                              name=f"stp{l}_{m}")
                nc.tensor.matmul(stp[:], lhsT=mask_sb[:, m:m + 1],
                                 rhs=pre_sb[:, m, :], start=True, stop=True)
                nc.vector.tensor_add(statacc[0:1, 0:512], statacc[0:1, 0:512], stp[:])
                stp2 = ps.tile([1, 512], F32, tag="stp", bufs=1,
                               name=f"stp2{l}_{m}")
                nc.tensor.matmul(stp2[:], lhsT=mask_sb[:, m:m + 1],
                                 rhs=sq[:], start=True, stop=True)
                nc.vector.tensor_add(statacc[0:1, 512:1024],
                                     statacc[0:1, 512:1024], stp2[:])

            # -- BN stats AllReduce
            arin = dram.tile([1, 1024], F32, tag="arin", name=f"arin{l}")
            arout_d = dram.tile([1, 1024], F32, tag="arout",
                                addr_space="Shared", name=f"arout{l}")
            nc.sync.dma_start(out=arin[:], in_=statacc[:])
            nc.gpsimd.collective_compute(
                "AllReduce", mybir.AluOpType.add, replica_groups=rg,
                ins=[arin[:].opt()], outs=[arout_d[:].opt()])
            aro = sbs.tile([1, 1024], F32, tag="aro", bufs=1, name=f"aro{l}")
            nc.sync.dma_start(out=aro[:], in_=arout_d[:])

            # A = gamma * rstd ; Bb = beta - mu * A   (rows: [A; Bb])
            mu = sbs.tile([1, 512], F32, tag="mu", bufs=1, name=f"mu{l}")
            nc.scalar.activation(mu[:], aro[0:1, 0:512], AF.Copy, scale=1.0 / N)
            ex2 = sbs.tile([1, 512], F32, tag="ex2", bufs=1, name=f"ex2{l}")
            nc.scalar.activation(ex2[:], aro[0:1, 512:1024], AF.Copy,
                                 scale=1.0 / N)
            var = sbs.tile([1, 512], F32, tag="var", bufs=1, name=f"var{l}")
            nc.vector.tensor_mul(var[:], mu[:], mu[:])
            nc.vector.tensor_sub(var[:], ex2[:], var[:])
            stdt = sbs.tile([1, 512], F32, tag="stdt", bufs=1, name=f"stdt{l}")
            nc.scalar.activation(stdt[:], var[:], AF.Sqrt, bias=BN_EPS)
            rstd = sbs.tile([1, 512], F32, tag="rstd", bufs=1, name=f"rstd{l}")
            nc.vector.reciprocal(rstd[:], stdt[:])
            ab = sbs.tile([2, 512], F32, tag="ab", bufs=1, name=f"ab{l}")
            nc.vector.tensor_mul(ab[0:1, :], gam_sb[0:1, l * 512:(l + 1) * 512],
                                 rstd[:])
            tmB = sbs.tile([1, 512], F32, tag="tmB", bufs=1, name=f"tmB{l}")
            nc.vector.tensor_mul(tmB[:], mu[:], ab[0:1, :])
            bbrow = sbs.tile([1, 512], F32, tag="bbrow", bufs=1,
                             name=f"bbrow{l}")
            nc.vector.tensor_sub(bbrow[:], bet_sb[0:1, l * 512:(l + 1) * 512],
                                 tmB[:])
            nc.sync.dma_start(out=ab[1:2, :], in_=bbrow[:])

            abT = sbs.tile([128, 4, 2], F32, tag="abT", name=f"abT{l}")
            for kc in range(4):
                tp = ps.tile([128, 2], F32, tag="den", bufs=1,
                             name=f"abtp{l}_{kc}")
                nc.tensor.transpose(tp[:], ab[:, kc * 128:(kc + 1) * 128],
                                    identf[0:2, 0:2])
                nc.vector.tensor_copy(out=abT[:, kc, :], in_=tp[:])

            # -- h_next = relu(A*pre + Bb) in feature-major; pool
            h_nxt = sbh.tile([128, 4, NLOC], BF, tag="h", name=f"h{l + 1}")
            poolp = ps.tile([8, 512], F32, tag="acc", bufs=2,
                            name=f"poolp{l}")
            for m in range(NT):
                hnm = sbm.tile([128, 512], BF, tag="hnm", bufs=2,
                               name=f"hnm{l}_{m}")
                for kc in range(4):
                    tp1 = ps.tile([128, 128], F32, tag="qd", bufs=2,
                                  name=f"tp1{l}_{m}_{kc}")
                    nc.tensor.transpose(
                        tp1[:], pre_sb[:, m, kc * 128:(kc + 1) * 128],
                        identf[:])
                    nc.scalar.activation(
                        h_nxt[:, kc, m * 128:(m + 1) * 128], tp1[:], AF.Relu,
                        scale=abT[:, kc, 0:1], bias=abT[:, kc, 1:2])
                    tp2 = ps.tile([128, 128], BF, tag="root", bufs=1,
                                  name=f"tp2{l}_{m}_{kc}")
                    nc.tensor.transpose(
                        tp2[:], h_nxt[:, kc, m * 128:(m + 1) * 128],
                        identb[:])
                    nc.scalar.activation(hnm[:, kc * 128:(kc + 1) * 128],
                                         tp2[:], AF.Copy)
                nc.tensor.matmul(poolp[:],
                                 lhsT=spool_sb[:, m * GPC:(m + 1) * GPC],
                                 rhs=hnm[:], start=(m == 0), stop=(m == NT - 1))
            pool_sb = sbs.tile([GPC, 512], F32, tag="poolsb", bufs=1,
                               name=f"pool{l}")
            nc.scalar.activation(pool_sb[:], poolp[:], AF.Identity,
                                 scale=cntr_sb[:, 0:1])
            nc.sync.dma_start(out=OUT[:, l * 512:(l + 1) * 512],
                              in_=pool_sb[:])

            h_cur = h_nxt

    return nc


def _host_shard(x, edge_index, batch):
    """Build all per-core host-side index/constant arrays."""
    batch = np.asarray(batch)
    src = np.asarray(edge_index[0])
    dst = np.asarray(edge_index[1])

    node_start = np.searchsorted(batch, np.arange(0, B, GPC))
    node_end = np.searchsorted(batch, np.arange(GPC - 1, B, GPC), side="right")
    nloc = node_end - node_start
    assert (nloc <= NLOC).all(), f"core node overflow {nloc}"

    core_of_node = batch // GPC               # [N]
    local_of_node = np.arange(N) - node_start[core_of_node]
    grow_of_node = core_of_node * NLOC + local_of_node  # global KV row

    ec = core_of_node[dst]
    ld = local_of_node[dst]

    idx16 = np.zeros((NCORE, 128, NGA * (GI // 16)), np.int16)
    dstc = np.full((NCORE, 128, CHUNKS), -1.0, np.float32)
    mask = np.zeros((NCORE, 128, NT), np.float32)
    spool = np.zeros((NCORE, 128, NT * GPC), np.float32)
    cntr = np.zeros((NCORE, GPC, 1), np.float32)
    xT = np.zeros((NCORE, 128, NLOC), np.float32)

    x = np.asarray(x)
    for c in range(NCORE):
        ns, nl = node_start[c], nloc[c]
        xT[c, :, :nl] = x[ns:ns + nl].T
        mask[c].reshape(-1)[:nl] = 0.0  # placeholder, fixed below
        m2 = np.zeros(NLOC, np.float32)
        m2[:nl] = 1.0
        mask[c] = m2.reshape(NT, 128).T
        # pooling selector + counts
        gl = batch[ns:ns + nl] - c * GPC   # local graph id per local node
        sp = np.zeros((NLOC, GPC), np.float32)
        sp[np.arange(nl), gl] = 1.0
        spool[c] = sp.reshape(NT, 128, GPC).transpose(1, 0, 2).reshape(
            128, NT * GPC)
        cnt = sp.sum(axis=0)
        cntr[c, :, 0] = 1.0 / np.maximum(cnt, 1.0)

        # edges of this core, sorted by local dst
        eids = np.nonzero(ec == c)[0]
        order = np.argsort(ld[eids], kind="stable")
        eids = eids[order]
        lds = ld[eids]
        srows = grow_of_node[src[eids]]
        blk = lds // 128
        # slot packing
        slot_src = np.zeros(SLOTS, np.int64)
        slot_dst = np.full(SLOTS, -1.0, np.float32)
        bc = np.bincount(blk, minlength=NT)
        assert (bc <= ES).all(), f"edge block overflow {bc.max()}"
        pos = 0
        for b_ in range(NT):
            n_ = bc[b_]
            sl = b_ * ES
            slot_src[sl:sl + n_] = srows[pos:pos + n_]
            slot_dst[sl:sl + n_] = (lds[pos:pos + n_] % 128).astype(np.float32)
            pos += n_
        # gather indices, wrapped in 16 partitions, replicated to 128
        w = slot_src.reshape(NGA, GI // 16, 16)
        for r in range(8):
            idx16[c, r * 16:(r + 1) * 16, :] = w.transpose(2, 0, 1).reshape(
                16, -1)
        dstc[c] = slot_dst.reshape(CHUNKS, 128).T

    dstr = dstc.transpose(0, 2, 1).copy()
    return (node_start, nloc, idx16, dstc, dstr, mask, spool, cntr, xT)


def kernel(x, edge_index, batch, W0_q, b0_q, W0_k, b0_k, W0_v, b0_v,
           W0_s, b0_s, Wq, bq, Wk, bk, Wv, bv, Ws, bs, gamma, beta):
    from concourse.bass_utils import run_bass_kernel_spmd

    (node_start, nloc, idx16, dstc, dstr, mask, spool, cntr, xT) = \
        _host_shard(x, edge_index, batch)

    # weights, packed (shared by all cores)
    W0a = np.concatenate([np.asarray(W0_q), np.asarray(W0_k),
                          np.asarray(W0_v), np.asarray(W0_s)], axis=1)  # [128, 2048]
    WRa = np.zeros((7 * 2048, 512), np.float32)
    Wstack = [np.asarray(Wq), np.asarray(Wk), np.asarray(Wv), np.asarray(Ws)]
    for li in range(7):
        for pr in range(4):
            for kc in range(4):
                r0 = li * 2048 + pr * 512 + kc * 128
                WRa[r0:r0 + 128] = Wstack[pr][li][kc * 128:(kc + 1) * 128, :]
    BIASa = np.zeros((8, 2048), np.float32)
    BIASa[0] = np.concatenate([np.asarray(b0_q), np.asarray(b0_k),
                               np.asarray(b0_v), np.asarray(b0_s)])
    bstack = [np.asarray(bq), np.asarray(bk), np.asarray(bv), np.asarray(bs)]
    for li in range(7):
        BIASa[li + 1] = np.concatenate([bstack[pr][li] for pr in range(4)])

    iota_f = np.tile(np.arange(128, dtype=np.float32)[None, :], (128, 1))
    iota_c = np.arange(128, dtype=np.float32)[:, None]
    ones1 = np.ones((1, 128), np.float32)
    ident = np.eye(128, dtype=np.float32)

    common = {
        "W0": _to_bf(W0a), "WR": _to_bf(WRa), "BIAS": _to_bf(BIASa.reshape(1, -1)),
        "GAM": np.asarray(gamma, np.float32).reshape(1, -1),
        "BET": np.asarray(beta, np.float32).reshape(1, -1),
        "IOTAF": _to_bf(iota_f), "IOTAC": iota_c,
        "ONES1": _to_bf(ones1), "IDENTF": ident, "IDENTB": _to_bf(ident),
    }
    in_maps = []
    for c in range(NCORE):
        in_maps.append(dict(
            common,
            XT=_to_bf(xT[c]), IDX=idx16[c],
            DSTC=dstc[c],
            MASK=mask[c], SPOOL=_to_bf(spool[c]), CNTR=cntr[c],
        ))

    nc = _build_nc()
    nc.compile()
    res = run_bass_kernel_spmd(nc, in_maps, list(range(NCORE)))
    out = np.zeros((B, L * 512), np.float32)
    for c in range(NCORE):
        out[c * GPC:(c + 1) * GPC] = res.results[c]["POOLED"]
    return out


if __name__ == "__main__":
    pass



# revision 6
# speedup vs baseline: 7.4706x; 7.4706x over previous
"""GraphTransformerEncoder (8-layer TransformerConv + BN + ReLU + mean-pool)
on 8 Trainium2 NeuronCores via Bass/Tile.

Sharding: graph-parallel. Core c owns graphs [8c, 8c+8) -> a contiguous node
range (batch is sorted). Edges are owned by the core of their dst node, sorted
by dst, and packed into fixed-stride per-128-node-block slots. Per layer each
core computes Q/K/V/root projections for its nodes, AllGathers the K|V table
(bf16, node-rows), DMA-gathers K|V rows for its edges' src nodes, computes the
edge softmax via segment-indicator matmuls, applies BN (global stats via a tiny
AllReduce) + ReLU, and mean-pools its graphs.
"""

import numpy as np
import ml_dtypes

import concourse.bass as bass
import concourse.bacc as bacc
import concourse.mybir as mybir
import concourse.tile as tile
from concourse import library_config
from contextlib import ExitStack

BF = mybir.dt.bfloat16
F32 = mybir.dt.float32
I16 = mybir.dt.int16
AF = mybir.ActivationFunctionType

# problem constants
N, E, F, H, C, L, B = 10000, 160000, 128, 8, 64, 8, 64
D = H * C  # 512
BN_EPS = 1e-5

# sharding constants
NCORE = 8
GPC = B // NCORE        # graphs per core = 8
NT = 11                 # node blocks (128) per core
NLOC = NT * 128         # 1408 node slots per core
NCH = 20                # edge chunks (128) per node block
ES = NCH * 128          # 2560 edge slots per node block
CHUNKS = NT * NCH       # 220
SLOTS = NT * ES         # 28160 edge slots per core
GI = 512                # indices per dma_gather (4 chunks)
GPB = ES // GI          # gathers per block = 5
NGA = SLOTS // GI       # gathers per core = 55
KVROWS = NCORE * NLOC   # 11264 rows in the all-gathered KV table


def _to_bf(a):
    return np.asarray(a, dtype=np.float32).astype(ml_dtypes.bfloat16)


def _build_nc(sim_local=False):
    # sim_local=True replaces collectives with local DMAs of the same
    # volume so the (collective-incapable) TimelineSim can profile it.
    nc = bacc.Bacc("TRN2", num_devices=NCORE,
                  target_bir_lowering=False, debug=False)
    rg = [list(range(NCORE))]

    # ---- I/O -----------------------------------------------------------
    XT = nc.dram_tensor("XT", [128, NLOC], BF, kind="ExternalInput")
    W0 = nc.dram_tensor("W0", [128, 4 * 512], BF, kind="ExternalInput")
    WR = nc.dram_tensor("WR", [7 * 2048, 512], BF, kind="ExternalInput")
    BIAS = nc.dram_tensor("BIAS", [1, 8 * 2048], BF, kind="ExternalInput")
    GAM = nc.dram_tensor("GAM", [1, 8 * 512], F32, kind="ExternalInput")
    BET = nc.dram_tensor("BET", [1, 8 * 512], F32, kind="ExternalInput")
    IDX = nc.dram_tensor("IDX", [128, NGA * (GI // 16)], I16, kind="ExternalInput")
    DSTC = nc.dram_tensor("DSTC", [128, CHUNKS], F32, kind="ExternalInput")
    IOTAF = nc.dram_tensor("IOTAF", [128, 128], BF, kind="ExternalInput")
    IOTAC = nc.dram_tensor("IOTAC", [128, 1], F32, kind="ExternalInput")
    ONES1 = nc.dram_tensor("ONES1", [1, 128], BF, kind="ExternalInput")
    IDENTF = nc.dram_tensor("IDENTF", [128, 128], F32, kind="ExternalInput")
    IDENTB = nc.dram_tensor("IDENTB", [128, 128], BF, kind="ExternalInput")
    MASK = nc.dram_tensor("MASK", [128, NT], F32, kind="ExternalInput")
    SPOOL = nc.dram_tensor("SPOOL", [128, NT * GPC], BF, kind="ExternalInput")
    CNTR = nc.dram_tensor("CNTR", [GPC, 1], F32, kind="ExternalInput")
    OUT = nc.dram_tensor("POOLED", [GPC, L * 512], F32, kind="ExternalOutput")

    with tile.TileContext(nc) as tc, ExitStack() as ctx:
        sb1 = ctx.enter_context(tc.tile_pool(name="sb1", bufs=1))
        sbh = ctx.enter_context(tc.tile_pool(name="sbh", bufs=2))
        sbw = ctx.enter_context(tc.tile_pool(name="sbw", bufs=2))
        sbs = ctx.enter_context(tc.tile_pool(name="sbs", bufs=3))
        sbg = ctx.enter_context(tc.tile_pool(name="sbg", bufs=3))
        sbm = ctx.enter_context(tc.tile_pool(name="sbm", bufs=2))
        ps = ctx.enter_context(tc.tile_pool(name="ps", bufs=1, space="PSUM"))
        dram = ctx.enter_context(tc.tile_pool(name="dram", bufs=2, space="DRAM"))

        def load1(src, shape, dtype, name):
            t = sb1.tile(shape, dtype, name=name)
            nc.sync.dma_start(out=t[:], in_=src[:])
            return t

        iota_f = load1(IOTAF, [128, 128], BF, "iota_f")
        iota_c = load1(IOTAC, [128, 1], F32, "iota_c")
        ones1 = load1(ONES1, [1, 128], BF, "ones1")
        identf = load1(IDENTF, [128, 128], F32, "identf")
        identb = load1(IDENTB, [128, 128], BF, "identb")
        idx_sb = load1(IDX, [128, NGA * (GI // 16)], I16, "idx_sb")
        dstc_sb = load1(DSTC, [128, CHUNKS], F32, "dstc_sb")
        mask_sb = load1(MASK, [128, NT], F32, "mask_sb")
        spool_sb = load1(SPOOL, [128, NT * GPC], BF, "spool_sb")
        cntr_sb = load1(CNTR, [GPC, 1], F32, "cntr_sb")
        gam_sb = load1(GAM, [1, 8 * 512], F32, "gam_sb")
        bet_sb = load1(BET, [1, 8 * 512], F32, "bet_sb")

        czero = sb1.tile([128, 1], F32, name="czero")
        nc.vector.memset(czero[:], 0.0)
        ceps = sb1.tile([128, 1], F32, name="ceps")
        nc.vector.memset(ceps[:], BN_EPS)
        nc.const_aps.aps[(F32, 0.0)] = czero[:]
        nc.const_aps.aps[(F32, BN_EPS)] = ceps[:]

        nc.gpsimd.load_library(library_config.mlp)

        h_cur = sbh.tile([128, 4, NLOC], BF, tag="h", name="h0")
        nc.sync.dma_start(out=h_cur[:, 0, :], in_=XT[:, :])

        for l in range(L):
            KIN = 1 if l == 0 else 4

            # -- weights for this layer -> SBUF [128, 4*KIN, 512]
            w_sb = sbw.tile([128, 4 * KIN, 512], BF, tag="w", name=f"w{l}")
            if l == 0:
                nc.sync.dma_start(
                    out=w_sb[:], in_=W0[:, :].rearrange("p (c n) -> p c n", c=4))
            else:
                nc.sync.dma_start(
                    out=w_sb[:],
                    in_=WR[(l - 1) * 2048: l * 2048, :].rearrange(
                        "(c p) n -> p c n", p=128))

            bias_sb = sbs.tile([1, 2048], BF, tag="bias", bufs=2,
                               name=f"bias{l}")
            nc.sync.dma_start(out=bias_sb[:],
                              in_=BIAS[0:1, l * 2048:(l + 1) * 2048])
            kv_loc = dram.tile([NLOC, 1024], BF, tag="kvloc", name=f"kvloc{l}")
            kv_full = dram.tile([KVROWS, 1024], BF, tag="kvfull",
                                **({} if sim_local else
                                   {"addr_space": "Shared"}),
                                name=f"kvfull{l}")

            # -- projections Q, K, V  (root is computed per block later)
            Q_sb = sbm.tile([128, NT, 512], BF, tag="q", bufs=1, name=f"q{l}")
            for m in range(NT):
                kv_sb = sbg.tile([128, 1024], BF, tag="kv", name=f"kv{l}_{m}")
                for pr in range(3):  # 0=q 1=k 2=v
                    pp = ps.tile([128, 512], F32, tag="qd", bufs=2,
                                 name=f"pp{l}_{m}_{pr}")
                    for kc in range(KIN):
                        nc.tensor.matmul(
                            pp[:], lhsT=h_cur[:, kc, m * 128:(m + 1) * 128],
                            rhs=w_sb[:, pr * KIN + kc, :],
                            start=(kc == 0), stop=False)
                    nc.tensor.matmul(
                        pp[:], lhsT=ones1[:],
                        rhs=bias_sb[0:1, pr * 512:(pr + 1) * 512],
                        start=False, stop=True)
                    if pr == 0:
                        nc.scalar.activation(Q_sb[:, m, :], pp[:], AF.Copy)
                    else:
                        nc.scalar.activation(
                            kv_sb[:, (pr - 1) * 512: pr * 512], pp[:], AF.Copy)
                nc.sync.dma_start(out=kv_loc[m * 128:(m + 1) * 128, :],
                                  in_=kv_sb[:])

            # -- AllGather the K|V table
            if sim_local:
                for c in range(NCORE):
                    nc.sync.dma_start(
                        out=kv_full[c * NLOC:(c + 1) * NLOC, :], in_=kv_loc[:])
            else:
                nc.gpsimd.collective_compute(
                    "AllGather", mybir.AluOpType.bypass, replica_groups=rg,
                    ins=[kv_loc[:].opt()], outs=[kv_full[:].opt()])

            # -- edge stage
            statacc = sbm.tile([1, 1024], F32, tag="stat", bufs=1,
                               name=f"stat{l}")
            nc.vector.memset(statacc[:], 0.0)
            pre_sb = sbm.tile([128, NT, 512], F32, tag="pre", bufs=1,
                              name=f"pre{l}")

            for m in range(NT):
                # root projection for this block
                rt = ps.tile([128, 512], F32, tag="root", bufs=1,
                             name=f"rt{l}_{m}")
                for kc in range(KIN):
                    nc.tensor.matmul(
                        rt[:], lhsT=h_cur[:, kc, m * 128:(m + 1) * 128],
                        rhs=w_sb[:, 3 * KIN + kc, :],
                        start=(kc == 0), stop=False)
                nc.tensor.matmul(rt[:], lhsT=ones1[:],
                                 rhs=bias_sb[0:1, 3 * 512:4 * 512],
                                 start=False, stop=True)

                acc = ps.tile([128, 512], F32, tag="acc", bufs=2,
                              name=f"acc{l}_{m}")
                den = ps.tile([128, 8], F32, tag="den", bufs=1,
                              name=f"den{l}_{m}")
                for g in range(GPB):
                    gt = sbg.tile([128, GI // 128, 1024], BF, tag="g",
                                  name=f"gt{l}_{m}_{g}")
                    ga = m * GPB + g
                    nc.gpsimd.dma_gather(
                        gt[:], kv_full[:, :],
                        idx_sb[:, ga * (GI // 16):(ga + 1) * (GI // 16)],
                        GI, GI, 1024)
                    for cc in range(GI // 128):
                        ch = m * NCH + g * (GI // 128) + cc
                        first = (g == 0 and cc == 0)
                        last = (g == GPB - 1 and cc == GI // 128 - 1)

                        st_sb = sbs.tile([128, 128], BF, tag="st",
                                         name=f"st{l}_{ch}")
                        nc.gpsimd.tensor_scalar(
                            out=st_sb[:], in0=iota_f[:],
                            scalar1=dstc_sb[:, ch:ch + 1],
                            scalar2=None, op0=mybir.AluOpType.is_equal)
                        str_ps = ps.tile([128, 128], BF, tag="str", bufs=1,
                                         name=f"strp{l}_{ch}")
                        nc.tensor.transpose(str_ps[:], st_sb[:], identb[:])
                        s_sb = sbs.tile([128, 128], BF, tag="s",
                                        name=f"s{l}_{ch}")
                        nc.vector.tensor_copy(out=s_sb[:], in_=str_ps[:])

                        qd = ps.tile([128, 512], F32, tag="qd", bufs=2,
                                     name=f"qd{l}_{ch}")
                        nc.tensor.matmul(qd[:], lhsT=s_sb[:],
                                         rhs=Q_sb[:, m, :],
                                         start=True, stop=True)
                        prod = sbm.tile([128, 512], BF, tag="prod", bufs=2,
                                        name=f"prod{l}_{ch}")
                        nc.vector.tensor_mul(prod[:], qd[:],
                                             gt[:, cc, 0:512])
                        logit = sbs.tile([128, 8], F32, tag="lg",
                                         name=f"lg{l}_{ch}")
                        nc.vector.tensor_reduce(
                            logit[:],
                            prod[:].rearrange("p (h c) -> p h c", h=8),
                            mybir.AxisListType.X, mybir.AluOpType.add)
                        pf = sbs.tile([128, 8], F32, tag="pf",
                                      name=f"pf{l}_{ch}")
                        nc.scalar.activation(pf[:], logit[:], AF.Exp,
                                             scale=0.125)
                        pbf = sbs.tile([128, 8], BF, tag="p",
                                       name=f"p{l}_{ch}")
                        nc.vector.tensor_copy(out=pbf[:], in_=pf[:])
                        pv = sbm.tile([128, 512], BF, tag="pv", bufs=2,
                                      name=f"pv{l}_{ch}")
                        for h in range(H):
                            nc.scalar.activation(
                                pv[:, h * 64:(h + 1) * 64],
                                gt[:, cc, 512 + h * 64: 512 + (h + 1) * 64],
                                AF.Identity, scale=pf[:, h:h + 1])
                        nc.tensor.matmul(acc[:], lhsT=st_sb[:], rhs=pv[:],
                                         start=first, stop=last)
                        nc.tensor.matmul(den[:], lhsT=st_sb[:], rhs=pbf[:],
                                         start=first, stop=last)

                # block finalize: normalize, add root, stats
                dsb = sbs.tile([128, 8], F32, tag="dsb", name=f"dsb{l}_{m}")
                nc.scalar.activation(dsb[:], den[:], AF.Copy, bias=1e-16)
                rec = sbs.tile([128, 8], F32, tag="rec", name=f"rec{l}_{m}")
                nc.vector.reciprocal(rec[:], dsb[:])
                msg = sbm.tile([128, 512], F32, tag="msg", bufs=2,
                               name=f"msg{l}_{m}")
                for h in range(H):
                    nc.scalar.activation(
                        msg[:, h * 64:(h + 1) * 64],
                        acc[:, h * 64:(h + 1) * 64],
                        AF.Identity, scale=rec[:, h:h + 1])
                nc.vector.tensor_add(pre_sb[:, m, :], msg[:], rt[:])
                sq = sbm.tile([128, 512], F32, tag="sq", bufs=2,
                              name=f"sq{l}_{m}")
                nc.scalar.activation(sq[:], pre_sb[:, m, :], AF.Square)
                stp = ps.tile([1, 512], F32, tag="stp", bufs=1,
                              name=f"stp{l}_{m}")
                nc.tensor.matmul(stp[:], lhsT=mask_sb[:, m:m + 1],
                                 rhs=pre_sb[:, m, :], start=True, stop=True)
                nc.vector.tensor_add(statacc[0:1, 0:512], statacc[0:1, 0:512], stp[:])
                stp2 = ps.tile([1, 512], F32, tag="stp", bufs=1,
                               name=f"stp2{l}_{m}")
                nc.tensor.matmul(stp2[:], lhsT=mask_sb[:, m:m + 1],
                                 rhs=sq[:], start=True, stop=True)
                nc.vector.tensor_add(statacc[0:1, 512:1024],
                                     statacc[0:1, 512:1024], stp2[:])

            # -- BN stats AllReduce
            arin = dram.tile([1, 1024], F32, tag="arin", name=f"arin{l}")
            arout_d = dram.tile([1, 1024], F32, tag="arout",
                                **({} if sim_local else
                                   {"addr_space": "Shared"}),
                                name=f"arout{l}")
            nc.sync.dma_start(out=arin[:], in_=statacc[:])
            if sim_local:
                nc.sync.dma_start(out=arout_d[:], in_=arin[:])
            else:
                nc.gpsimd.collective_compute(
                    "AllReduce", mybir.AluOpType.add, replica_groups=rg,
                    ins=[arin[:].opt()], outs=[arout_d[:].opt()])
            aro = sbs.tile([1, 1024], F32, tag="aro", bufs=1, name=f"aro{l}")
            nc.sync.dma_start(out=aro[:], in_=arout_d[:])

            # A = gamma * rstd ; Bb = beta - mu * A   (rows: [A; Bb])
            mu = sbs.tile([1, 512], F32, tag="mu", bufs=1, name=f"mu{l}")
            nc.scalar.activation(mu[:], aro[0:1, 0:512], AF.Copy, scale=1.0 / N)
            ex2 = sbs.tile([1, 512], F32, tag="ex2", bufs=1, name=f"ex2{l}")
            nc.scalar.activation(ex2[:], aro[0:1, 512:1024], AF.Copy,
                                 scale=1.0 / N)
            var = sbs.tile([1, 512], F32, tag="var", bufs=1, name=f"var{l}")
            nc.vector.tensor_mul(var[:], mu[:], mu[:])
            nc.vector.tensor_sub(var[:], ex2[:], var[:])
            stdt = sbs.tile([1, 512], F32, tag="stdt", bufs=1, name=f"stdt{l}")
            nc.scalar.activation(stdt[:], var[:], AF.Sqrt, bias=BN_EPS)
            rstd = sbs.tile([1, 512], F32, tag="rstd", bufs=1, name=f"rstd{l}")
            nc.vector.reciprocal(rstd[:], stdt[:])
            ab = sbs.tile([2, 512], F32, tag="ab", bufs=1, name=f"ab{l}")
            nc.vector.tensor_mul(ab[0:1, :], gam_sb[0:1, l * 512:(l + 1) * 512],
                                 rstd[:])
            tmB = sbs.tile([1, 512], F32, tag="tmB", bufs=1, name=f"tmB{l}")
            nc.vector.tensor_mul(tmB[:], mu[:], ab[0:1, :])
            bbrow = sbs.tile([1, 512], F32, tag="bbrow", bufs=1,
                             name=f"bbrow{l}")
            nc.vector.tensor_sub(bbrow[:], bet_sb[0:1, l * 512:(l + 1) * 512],
                                 tmB[:])
            nc.sync.dma_start(out=ab[1:2, :], in_=bbrow[:])

            abT = sbs.tile([128, 4, 2], F32, tag="abT", name=f"abT{l}")
            for kc in range(4):
                tp = ps.tile([128, 2], F32, tag="den", bufs=1,
                             name=f"abtp{l}_{kc}")
                nc.tensor.transpose(tp[:], ab[:, kc * 128:(kc + 1) * 128],
                                    identf[0:2, 0:2])
                nc.vector.tensor_copy(out=abT[:, kc, :], in_=tp[:])

            # -- h_next = relu(A*pre + Bb) in feature-major; pool
            h_nxt = sbh.tile([128, 4, NLOC], BF, tag="h", name=f"h{l + 1}")
            poolp = ps.tile([8, 512], F32, tag="acc", bufs=2,
                            name=f"poolp{l}")
            for m in range(NT):
                hnm = sbm.tile([128, 512], BF, tag="hnm", bufs=2,
                               name=f"hnm{l}_{m}")
                for kc in range(4):
                    tp1 = ps.tile([128, 128], F32, tag="qd", bufs=2,
                                  name=f"tp1{l}_{m}_{kc}")
                    nc.tensor.transpose(
                        tp1[:], pre_sb[:, m, kc * 128:(kc + 1) * 128],
                        identf[:])
                    nc.scalar.activation(
                        h_nxt[:, kc, m * 128:(m + 1) * 128], tp1[:], AF.Relu,
                        scale=abT[:, kc, 0:1], bias=abT[:, kc, 1:2])
                    tp2 = ps.tile([128, 128], BF, tag="root", bufs=1,
                                  name=f"tp2{l}_{m}_{kc}")
                    nc.tensor.transpose(
                        tp2[:], h_nxt[:, kc, m * 128:(m + 1) * 128],
                        identb[:])
                    nc.scalar.activation(hnm[:, kc * 128:(kc + 1) * 128],
                                         tp2[:], AF.Copy)
                nc.tensor.matmul(poolp[:],
                                 lhsT=spool_sb[:, m * GPC:(m + 1) * GPC],
                                 rhs=hnm[:], start=(m == 0), stop=(m == NT - 1))
            pool_sb = sbs.tile([GPC, 512], F32, tag="poolsb", bufs=1,
                               name=f"pool{l}")
            nc.scalar.activation(pool_sb[:], poolp[:], AF.Identity,
                                 scale=cntr_sb[:, 0:1])
            nc.sync.dma_start(out=OUT[:, l * 512:(l + 1) * 512],
                              in_=pool_sb[:])

            h_cur = h_nxt

    return nc


def _host_shard(x, edge_index, batch):
    """Build all per-core host-side index/constant arrays."""
    batch = np.asarray(batch)
    src = np.asarray(edge_index[0])
    dst = np.asarray(edge_index[1])

    node_start = np.searchsorted(batch, np.arange(0, B, GPC))
    node_end = np.searchsorted(batch, np.arange(GPC - 1, B, GPC), side="right")
    nloc = node_end - node_start
    assert (nloc <= NLOC).all(), f"core node overflow {nloc}"

    core_of_node = batch // GPC               # [N]
    local_of_node = np.arange(N) - node_start[core_of_node]
    grow_of_node = core_of_node * NLOC + local_of_node  # global KV row

    ec = core_of_node[dst]
    ld = local_of_node[dst]

    idx16 = np.zeros((NCORE, 128, NGA * (GI // 16)), np.int16)
    dstc = np.full((NCORE, 128, CHUNKS), -1.0, np.float32)
    mask = np.zeros((NCORE, 128, NT), np.float32)
    spool = np.zeros((NCORE, 128, NT * GPC), np.float32)
    cntr = np.zeros((NCORE, GPC, 1), np.float32)
    xT = np.zeros((NCORE, 128, NLOC), np.float32)

    x = np.asarray(x)
    for c in range(NCORE):
        ns, nl = node_start[c], nloc[c]
        xT[c, :, :nl] = x[ns:ns + nl].T
        mask[c].reshape(-1)[:nl] = 0.0  # placeholder, fixed below
        m2 = np.zeros(NLOC, np.float32)
        m2[:nl] = 1.0
        mask[c] = m2.reshape(NT, 128).T
        # pooling selector + counts
        gl = batch[ns:ns + nl] - c * GPC   # local graph id per local node
        sp = np.zeros((NLOC, GPC), np.float32)
        sp[np.arange(nl), gl] = 1.0
        spool[c] = sp.reshape(NT, 128, GPC).transpose(1, 0, 2).reshape(
            128, NT * GPC)
        cnt = sp.sum(axis=0)
        cntr[c, :, 0] = 1.0 / np.maximum(cnt, 1.0)

        # edges of this core, sorted by local dst
        eids = np.nonzero(ec == c)[0]
        order = np.argsort(ld[eids], kind="stable")
        eids = eids[order]
        lds = ld[eids]
        srows = grow_of_node[src[eids]]
        blk = lds // 128
        # slot packing
        slot_src = np.zeros(SLOTS, np.int64)
        slot_dst = np.full(SLOTS, -1.0, np.float32)
        bc = np.bincount(blk, minlength=NT)
        assert (bc <= ES).all(), f"edge block overflow {bc.max()}"
        pos = 0
        for b_ in range(NT):
            n_ = bc[b_]
            sl = b_ * ES
            slot_src[sl:sl + n_] = srows[pos:pos + n_]
            slot_dst[sl:sl + n_] = (lds[pos:pos + n_] % 128).astype(np.float32)
            pos += n_
        # gather indices, wrapped in 16 partitions, replicated to 128
        w = slot_src.reshape(NGA, GI // 16, 16)
        for r in range(8):
            idx16[c, r * 16:(r + 1) * 16, :] = w.transpose(2, 0, 1).reshape(
                16, -1)
        dstc[c] = slot_dst.reshape(CHUNKS, 128).T

    dstr = dstc.transpose(0, 2, 1).copy()
    return (node_start, nloc, idx16, dstc, dstr, mask, spool, cntr, xT)


def kernel(x, edge_index, batch, W0_q, b0_q, W0_k, b0_k, W0_v, b0_v,
           W0_s, b0_s, Wq, bq, Wk, bk, Wv, bv, Ws, bs, gamma, beta):
    from concourse.bass_utils import run_bass_kernel_spmd

    (node_start, nloc, idx16, dstc, dstr, mask, spool, cntr, xT) = \
        _host_shard(x, edge_index, batch)

    # weights, packed (shared by all cores)
    W0a = np.concatenate([np.asarray(W0_q), np.asarray(W0_k),
                          np.asarray(W0_v), np.asarray(W0_s)], axis=1)  # [128, 2048]
    WRa = np.zeros((7 * 2048, 512), np.float32)
    Wstack = [np.asarray(Wq), np.asarray(Wk), np.asarray(Wv), np.asarray(Ws)]
    for li in range(7):
        for pr in range(4):
            for kc in range(4):
                r0 = li * 2048 + pr * 512 + kc * 128
                WRa[r0:r0 + 128] = Wstack[pr][li][kc * 128:(kc + 1) * 128, :]
    BIASa = np.zeros((8, 2048), np.float32)
    BIASa[0] = np.concatenate([np.asarray(b0_q), np.asarray(b0_k),
                               np.asarray(b0_v), np.asarray(b0_s)])
    bstack = [np.asarray(bq), np.asarray(bk), np.asarray(bv), np.asarray(bs)]
    for li in range(7):
        BIASa[li + 1] = np.concatenate([bstack[pr][li] for pr in range(4)])

    iota_f = np.tile(np.arange(128, dtype=np.float32)[None, :], (128, 1))
    iota_c = np.arange(128, dtype=np.float32)[:, None]
    ones1 = np.ones((1, 128), np.float32)
    ident = np.eye(128, dtype=np.float32)

    common = {
        "W0": _to_bf(W0a), "WR": _to_bf(WRa), "BIAS": _to_bf(BIASa.reshape(1, -1)),
        "GAM": np.asarray(gamma, np.float32).reshape(1, -1),
        "BET": np.asarray(beta, np.float32).reshape(1, -1),
        "IOTAF": _to_bf(iota_f), "IOTAC": iota_c,
        "ONES1": _to_bf(ones1), "IDENTF": ident, "IDENTB": _to_bf(ident),
    }
    in_maps = []
    for c in range(NCORE):
        in_maps.append(dict(
            common,
            XT=_to_bf(xT[c]), IDX=idx16[c],
            DSTC=dstc[c],
            MASK=mask[c], SPOOL=_to_bf(spool[c]), CNTR=cntr[c],
        ))

    nc = _build_nc()
    nc.compile()
    res = run_bass_kernel_spmd(nc, in_maps, list(range(NCORE)))
    out = np.zeros((B, L * 512), np.float32)
    for c in range(NCORE):
        out[c * GPC:(c + 1) * GPC] = res.results[c]["POOLED"]
    return out


if __name__ == "__main__":
    pass

